# revision 17
# baseline (speedup 1.0000x reference)
"""CropRoi (crop + adaptive max pool 3D) Trainium2 kernel.

kernel(**inputs) takes the FULL inputs from setup_inputs() and returns
the FULL [N, C, 7, 7, 7] float32 output, distributing over 8
NeuronCores.

Strategy: the host computes each proposal's crop window (bit-exact
int mirror of the reference math), permutes its axes, splits the
last-pooled axis symmetrically across the two 64-partition halves
(A = output bins 0-3 from x[0:lenA], B = bins 3-6 from x[s3:s3+lenA];
both halves are exactly lenA=ceil(4L/7) wide), and packs all windows
bf16 into one [128, W] DRAM buffer per core. The device runs the
separable adaptive max-pool as a minimal set of affine-run
tensor_max/copy instructions (bf16, DVE 2x mode; strided-innermost
APs avoided via a per-proposal storage-variant choice), with the
last axis pooled once per group of same-L3 proposals. Outputs leave
as two tight bf16 DMAs per group; the host reassembles and upcasts.
"""
import sys

sys.path.insert(0, "/opt/trn_rl_repo")

import numpy as np
import ml_dtypes


R = 7
SCALE = 4
B, C, FS = 4, 64, 32
N = 96
DIMS_MAX = (32, 32, 32)
N_CORES = 8
MAX_GROUP = 6

# engine cost model (ns), calibrated on HW probes:
#   bf16 tensor_max packed-inner: 2x mode ~0.57 ns/elem, fixed ~60
#   bf16 tensor_copy packed-inner: ~0.34 ns/elem (4x)
#   strided/degenerate innermost: ~4.2 ns/elem (!!)
#   ACT copy: ~0.83 ns/elem, ~370 fixed, one-time table load 1283
DVE_FIX = 200.0
MAX_EL_P, MAX_EL_S = 0.52, 1.6
CP_EL_P, CP_EL_S = 0.30, 1.6
ACT_FIX, ACT_EL = 420.0, 0.83


def max_cost(fd, packed):
    return DVE_FIX + fd * (MAX_EL_P if packed else MAX_EL_S)


def copy_cost_dve(fd, packed):
    return DVE_FIX + fd * (CP_EL_P if packed else CP_EL_S)


def copy_cost_act(fd):
    return ACT_FIX + fd * ACT_EL


def proposal_params(proposals):
    out = []
    f32 = np.float32
    for p in np.asarray(proposals, dtype=np.float32):
        b = int(np.int32(p[0]))
        center, side = p[2:5].astype(f32), p[5:8].astype(f32)
        lo = (center - side / f32(2.0)) / f32(SCALE)
        hi = (center + side / f32(2.0)) / f32(SCALE)
        c0 = np.floor(lo).astype(np.int32)
        c1 = np.ceil(hi).astype(np.int32)
        c0 = np.maximum(c0, 0)
        c1 = np.minimum(c1, np.array(DIMS_MAX, np.int32))
        out.append((b, tuple(int(x) for x in c0), tuple(int(x) for x in c1)))
    return out


def axis_bins(L):
    return [((i * L) // R, ((i + 1) * L + R - 1) // R) for i in range(R)]


def class_runs(items):
    """items: sorted list of (dst_i, src_s). Returns affine runs
    [(i0, s0, di, ds, cnt)]."""
    runs = []
    k = 0
    while k < len(items):
        i0, s0 = items[k]
        j = k + 1
        di = ds = None
        while j < len(items):
            pi, ps = items[j - 1]
            ci, cs = items[j]
            d_i, d_s = ci - pi, cs - ps
            if di is None:
                di, ds = d_i, d_s
            elif (d_i, d_s) != (di, ds):
                break
            j += 1
        if di is None:
            di, ds = 1, 1
        runs.append((i0, s0, di, ds, j - k))
        k = j
    return runs


def axis_ops(bins):
    """Ops to pool a dim with the given bins list [(s,e)] -> 7 outputs.
    Returns list of ('max2'|'max3'|'copy', runs) where runs are affine
    (i0, s0, di, ds, cnt). max2: dst=max(src[s],src[s+1]);
    max3: additionally fold src[s+2]. copy: dst=src[s]."""
    by_len = {}
    for i, (s, e) in enumerate(bins):
        by_len.setdefault(e - s, []).append((i, s))
    ops = []
    for ln, items in sorted(by_len.items()):
        kind = {1: "copy", 2: "max2", 3: "max3"}[ln]
        ops.append((kind, class_runs(items)))
    return ops


def stage_ops(bins, fd_unit, inner_sliced):
    """(kind, fd, packed) per emitted run for pooling with `bins`.
    fd_unit = free elems per unit of the sliced dim; inner_sliced =
    whether the sliced dim is the innermost (packed requires di=ds=1,
    cnt>=2 there)."""
    out = []
    for kind, runs in axis_ops(bins):
        for (i0, s0, di, ds, cnt) in runs:
            packed = (not inner_sliced) or (di == 1 and ds == 1 and cnt >= 2)
            out.append((kind, cnt * fd_unit, packed))
    return out


def ops_cost(ops):
    tot = 0.0
    for kind, fd, packed in ops:
        if kind == "copy":
            tot += min(copy_cost_dve(fd, packed), copy_cost_act(fd))
        else:
            n = 2 if kind == "max3" else 1
            tot += n * max_cost(fd, packed)
    return tot


def stage3_opsets(binsA, binsB, fd_unit, inner_sliced):
    """Merged/A-only/B-only op lists for stage 3 (shared-half merge)."""
    opsA = [(kind, run) for kind, runs in axis_ops(binsA) for run in runs]
    opsB = [(kind, run) for kind, runs in axis_ops(binsB) for run in runs]
    both = set(opsA) & set(opsB)

    def conv(ops):
        out = []
        for kind, (i0, s0, di, ds, cnt) in ops:
            packed = (not inner_sliced) or (di == 1 and ds == 1 and cnt >= 2)
            out.append((kind, cnt * fd_unit, packed))
        return out

    return (conv(sorted(both)) + conv([o for o in opsA if o not in both])
            + conv([o for o in opsB if o not in both]))


def stage3_cost(p, variant, g):
    if p.L3 == R:
        return 0.0
    return ops_cost(stage3_opsets(p.binsA, p.binsB, 49 * g,
                                  inner_sliced=(variant == "A")))


def split_info(L):
    """a3 symmetric split: half A = x[0:lenA] computes bins 0-3, half B =
    x[s3:s3+lenA] computes bins 3-6 (bin 3 duplicated). lenA = ceil(4L/7)
    for both halves — no padding. Returns (lenA, s3, binsA, binsB)."""
    bins = axis_bins(L)
    s3 = (3 * L) // R
    lenA = bins[3][1]                     # == L - s3 == ceil(4L/7)
    assert lenA == L - s3
    binsA = bins[0:4]                     # dst cols 0..3 = bins 0..3
    binsB = [(s - s3, e - s3) for (s, e) in bins[3:7]]  # dst 0..3 = bins 3..6
    return lenA, s3, binsA, binsB


class Prop:
    __slots__ = ("idx", "b", "c0", "c1", "order", "L", "L1", "L2", "L3",
                 "lenA", "s3", "binsA", "binsB", "w", "cost", "vcost",
                 "col", "slot", "variant")

    def __init__(self, idx, b, c0, c1):
        self.idx, self.b, self.c0, self.c1 = idx, b, c0, c1
        Lxyz = [c1[k] - c0[k] for k in range(3)]
        # Search: which axis to split (pooled last, grouped) x storage
        # variant 'A' ([L1,L2,lenA], split-axis innermost) vs 'B'
        # ([L1,lenA,L2], a2 innermost). Strided-innermost ops run ~8x
        # slower than packed, so pick by exact mode-aware cost.
        best = None
        for sp in range(3):
            rest = sorted([k for k in range(3) if k != sp],
                          key=lambda k: (-Lxyz[k], k))
            order = [rest[0], rest[1], sp]
            self.order = order
            self.L = [Lxyz[a] for a in order]
            self.L1, self.L2, self.L3 = self.L
            self.lenA, self.s3, self.binsA, self.binsB = split_info(self.L3)
            self.w = self.L1 * self.L2 * self.lenA
            for variant in (("A",) if self.L3 == R else ("A", "B")):
                self.variant = variant
                c = self._est()
                if best is None or c < best[0]:
                    best = (c, order, variant)
        _, order, variant = best
        self.order = order
        self.L = [Lxyz[a] for a in order]
        self.L1, self.L2, self.L3 = self.L
        self.lenA, self.s3, self.binsA, self.binsB = split_info(self.L3)
        self.variant = variant
        self.w = self.L1 * self.L2 * self.lenA   # cols in xin
        self.cost = self._est()
        self.vcost = self._vest()
        self.col = -1
        self.slot = -1

    def _cost12(self, variant):
        """Exact (mode-aware) cost of this proposal's stage-1+2 ops."""
        ops = []
        if self.L1 == R and self.L2 == R:
            ops.append(("copy", 49 * self.lenA, True))
        else:
            if self.L1 != R:
                ops += stage_ops(axis_bins(self.L1), self.L2 * self.lenA,
                                 inner_sliced=False)
            if self.L2 != R:
                ops += stage_ops(axis_bins(self.L2), R * self.lenA,
                                 inner_sliced=(variant == "B"))
        return ops_cost(ops)

    def _est(self):
        tot = self._cost12(self.variant)
        tot += stage3_cost(self, self.variant, 4) / 4.0
        # input DMA share (~0.36 ns per bf16 col across 128 partitions)
        tot += self.w * 0.15
        return tot

    def _ops(self):
        ops = []
        if self.L1 == R and self.L2 == R:
            ops.append(("copy", 49 * self.lenA, True))
        else:
            if self.L1 != R:
                ops += stage_ops(axis_bins(self.L1), self.L2 * self.lenA,
                                 inner_sliced=False)
            if self.L2 != R:
                ops += stage_ops(axis_bins(self.L2), R * self.lenA,
                                 inner_sliced=(self.variant == "B"))
        if self.L3 != R:
            ops += [(k, fd // 4, pk) for (k, fd, pk) in
                    stage3_opsets(self.binsA, self.binsB, 49 * 4,
                                  inner_sliced=(self.variant == "A"))]
        return ops

    def _vest(self):
        """DVE-load proxy: maxes always DVE; copies only if DVE-cheaper."""
        tot = 0.0
        for kind, fd, packed in self._ops():
            if kind == "copy":
                cd = copy_cost_dve(fd, packed)
                if cd <= copy_cost_act(fd):
                    tot += 0.5 * cd
            else:
                n = 2 if kind == "max3" else 1
                tot += n * max_cost(fd, packed)
        return tot


def plan(proposals):
    """Full plan: returns list of 8 core-plans. Each core-plan:
    dict(groups=[{L3, props:[Prop], slot0}], W=total cols, nslots)."""
    params = proposal_params(proposals)
    props = [Prop(i, b, c0, c1) for i, (b, c0, c1) in enumerate(params)]

    # global buckets by (L3, variant) — stage-3 structure must be
    # uniform within a group; split into groups <= MAX_GROUP
    buckets = {}
    for p in props:
        buckets.setdefault((p.L3, p.variant), []).append(p)
    groups = []
    for key in sorted(buckets):
        mem = sorted(buckets[key], key=lambda p: -p.cost)
        for s in range(0, len(mem), MAX_GROUP):
            g = mem[s:s + MAX_GROUP]
            groups.append({"L3": key[0], "props": g, "variant": key[1],
                           # + per-group output-DMA and per-proposal
                           # input-DMA issue overheads (ring time)
                           "cost": sum(p.cost for p in g) + 600
                                   + 250 * len(g),
                           "vcost": sum(p.vcost for p in g)})

    # LPT assignment of groups to cores on combined cost, then a
    # hill-climb refinement (move/swap single groups).
    cores = [{"groups": [], "cost": 0.0, "vcost": 0.0}
             for _ in range(N_CORES)]
    for g in sorted(groups, key=lambda g: -g["cost"]):
        c = min(cores, key=lambda c: c["cost"])
        c["groups"].append(g)
        c["cost"] += g["cost"]
        c["vcost"] += g["vcost"]

    def rebal():
        for _ in range(200):
            cores.sort(key=lambda c: -c["cost"])
            hi, lo = cores[0], cores[-1]
            best = None
            for g in hi["groups"]:
                nh, nl = hi["cost"] - g["cost"], lo["cost"] + g["cost"]
                if max(nh, nl) < hi["cost"]:
                    d = hi["cost"] - max(nh, nl)
                    if best is None or d > best[0]:
                        best = (d, g, None)
            for g in hi["groups"]:
                for h in lo["groups"]:
                    if g["cost"] <= h["cost"]:
                        continue
                    nh = hi["cost"] - g["cost"] + h["cost"]
                    nl = lo["cost"] + g["cost"] - h["cost"]
                    if max(nh, nl) < hi["cost"]:
                        d = hi["cost"] - max(nh, nl)
                        if best is None or d > best[0]:
                            best = (d, g, h)
            if best is None:
                return
            _, g, h = best
            hi["groups"].remove(g)
            hi["cost"] -= g["cost"]
            lo["groups"].append(g)
            lo["cost"] += g["cost"]
            if h is not None:
                lo["groups"].remove(h)
                lo["cost"] -= h["cost"]
                hi["groups"].append(h)
                hi["cost"] += h["cost"]

    rebal()

    # per-core ordering: smallest-bytes group first (its data arrives
    # fastest -> compute starts early), then by cost descending so the
    # final group has little compute and its output DMA issues early.
    for c in cores:
        gs = sorted(c["groups"], key=lambda g: sum(p.w for p in g["props"]))
        first, rest = gs[0], gs[1:]
        rest.sort(key=lambda g: -g["cost"])
        # put an identity group (L3==7: zero stage-3 ops) last when
        # available: the final output DMA then issues right after the
        # last stage-2 op instead of waiting on stage-3.
        ident = [g for g in rest if g["L3"] == R]
        if ident:
            gl = min(ident, key=lambda g: g["cost"])
            rest.remove(gl)
            rest.append(gl)
        elif first["L3"] == R and rest:
            # use the identity group as the tail instead of the head
            first2 = min(rest, key=lambda g: sum(p.w for p in g["props"]))
            rest.remove(first2)
            rest.append(first)
            first = first2
        c["groups"] = [first] + rest
        # order members to create runs of equal L2 (stage-2 batching);
        # first group by size so the first DMA lands fast.
        for gi, g in enumerate(c["groups"]):
            if gi == 0:
                g["props"].sort(key=lambda p: p.w)
            else:
                g["props"].sort(key=lambda p: (p.L2, p.w))
        col = 0
        slot = 0
        for g in c["groups"]:
            g["slot0"] = slot
            for p in g["props"]:
                p.col = col
                p.slot = slot
                col += p.w
                slot += 1
        c["W"] = col
        c["nslots"] = slot
    return cores


# ----------------------------------------------------------------------------
# Packing + reassembly (host)
# ----------------------------------------------------------------------------

def pack_core(f_bf16, core):
    """Build the [128, W] bf16 input for one core."""
    xin = np.zeros((128, core["W"]), dtype=f_bf16.dtype)
    for g in core["groups"]:
        for p in g["props"]:
            d0, h0, w0 = p.c0
            d1, h1, w1 = p.c1
            w = f_bf16[p.b, :, d0:d1, h0:h1, w0:w1]      # [C, Ld, Lh, Lw]
            w = np.transpose(w, [0] + [1 + a for a in p.order])  # [C,L1,L2,L3]
            A = w[..., :p.lenA]                            # [C, L1, L2, lenA]
            Bm = w[..., p.s3:p.s3 + p.lenA]
            if p.variant == "B":                           # [C, L1, lenA, L2]
                A = A.transpose(0, 1, 3, 2)
                Bm = Bm.transpose(0, 1, 3, 2)
            xin[0:64, p.col:p.col + p.w] = A.reshape(64, -1)
            xin[64:128, p.col:p.col + p.w] = Bm.reshape(64, -1)
    return xin


def unpack_core(core, out_dev, out_full):
    """out_dev: [nslots, 128, 196] (numpy float32 or bf16). Writes into
    out_full [N, C, 7,7,7] float32."""
    for g in core["groups"]:
        for p in g["props"]:
            a = np.asarray(out_dev[p.slot, 0:64], dtype=np.float32)
            b = np.asarray(out_dev[p.slot, 64:128], dtype=np.float32)
            if p.variant == "B":           # [64, 7, 4, 7] -> [64, 7, 7, 4]
                a = a.reshape(64, 7, 4, 7).transpose(0, 1, 3, 2)
                b = b.reshape(64, 7, 4, 7).transpose(0, 1, 3, 2)
            else:
                a = a.reshape(64, 7, 7, 4)
                b = b.reshape(64, 7, 7, 4)
            # A = bins 0-3, B = bins 3-6 (drop B's duplicated bin 3)
            asm = np.concatenate([a, b[..., 1:4]], axis=-1)  # [64,7,7,7]
            inv = [0, 0, 0]
            for storage_pos, ax in enumerate(p.order):
                inv[ax] = storage_pos
            asm = np.transpose(asm, [0] + [1 + inv[k] for k in range(3)])
            out_full[p.idx] = asm




BF16 = np.dtype(ml_dtypes.bfloat16)
ACT_TABLE_LOAD = 1283.0


class View:
    """Lightweight AP descriptor over a tile: dims[0] is partitions."""
    __slots__ = ("base", "off", "dims", "strides")

    def __init__(self, base, off, dims, strides):
        self.base = base          # a bass AP anchored at (part_lo, col 0)
        self.off = off            # element offset into the free space
        self.dims = list(dims)    # [n0, n1, ...] free dims (no partition)
        self.strides = list(strides)

    @classmethod
    def flat(cls, tile, part_lo, part_hi, col, dims):
        base = tile[part_lo:part_hi, 0:1]
        strides = []
        s = 1
        for d in reversed(dims):
            strides.append(s)
            s *= d
        strides.reverse()
        return cls(base, col, dims, strides)

    def slice(self, k, start, step, cnt):
        """Slice free dim k: start/step in dim units; step 0 = broadcast."""
        v = View(self.base, self.off, self.dims, self.strides)
        v.off += start * self.strides[k]
        v.dims[k] = cnt
        v.strides[k] = self.strides[k] * step
        return v

    def ap(self, bass):
        part = list(self.base.ap[0])
        ap = [part] + [[s, d] for s, d in zip(self.strides, self.dims)]
        return bass.AP(tensor=self.base.tensor,
                       offset=self.base.offset + self.off, ap=ap)


def _packed(*views):
    """True if every view's innermost AP dim is stride 1 with len >= 2."""
    for v in views:
        if v.strides[-1] != 1 or v.dims[-1] < 2:
            return False
    return True


class Sched:
    def __init__(self, nc, bass):
        self.nc = nc
        self.bass = bass
        self.dve = 0.0
        self.act = 0.0
        self.act_used = False

    def _fd(self, v):
        n = 1
        for d in v.dims:
            n *= d
        return n

    def tmax(self, dst, a, b):
        fd = self._fd(dst)
        self.dve += max_cost(fd, _packed(dst, a, b))
        self.nc.vector.tensor_max(dst.ap(self.bass), a.ap(self.bass),
                                  b.ap(self.bass))

    def copy(self, dst, src, late=False):
        fd = self._fd(dst)
        cd = copy_cost_dve(fd, _packed(dst, src))
        if late:
            # late copies gate the final output DMA; keep them off the
            # (slow, bursty) ACT engine entirely.
            self.dve += cd
            self.nc.vector.tensor_copy(dst.ap(self.bass), src.ap(self.bass))
            return
        ca = copy_cost_act(fd) + (0 if self.act_used else ACT_TABLE_LOAD)
        if self.act + ca <= self.dve + cd:
            self.act += ca
            self.act_used = True
            self.nc.scalar.copy(out=dst.ap(self.bass), in_=src.ap(self.bass))
        else:
            self.dve += cd
            self.nc.vector.tensor_copy(dst.ap(self.bass), src.ap(self.bass))


def emit_axis(sched, dst, src, k, bins, late=False):
    """Pool free dim k of src into dst (7 or 4 outputs) per bins."""
    for kind, runs in axis_ops(bins):
        for (i0, s0, di, ds, cnt) in runs:
            dv = dst.slice(k, i0, di if cnt > 1 else 1, cnt)
            if kind == "copy":
                sched.copy(dv, src.slice(k, s0, ds, cnt), late=late)
            else:
                sched.tmax(dv, src.slice(k, s0, ds, cnt),
                           src.slice(k, s0 + 1, ds, cnt))
                if kind == "max3":
                    sched.tmax(dv, dv, src.slice(k, s0 + 2, ds, cnt))


def build_core_program(core):
    import concourse.bacc as bacc
    import concourse.bass as bass
    import concourse.tile as tile
    from concourse import mybir

    W = core["W"]
    ns = core["nslots"]
    nc = bacc.Bacc("TRN2", target_bir_lowering=False, debug=False,
                   num_devices=1)
    xin_d = nc.dram_tensor("xin", [128, W], mybir.dt.bfloat16,
                           kind="ExternalInput")
    out_d = nc.dram_tensor("out", [ns, 128, 196], mybir.dt.bfloat16,
                           kind="ExternalOutput")

    with tile.TileContext(nc) as tc:
        with (
            tc.tile_pool(name="xin", bufs=1) as xin_pool,
            tc.tile_pool(name="x1", bufs=9) as x1_pool,
            tc.tile_pool(name="grp", bufs=5) as grp_pool,
            tc.tile_pool(name="go", bufs=5) as go_pool,
        ):
            xin_t = xin_pool.tile([128, W], mybir.dt.bfloat16, tag="xin",
                                  name="xin")
            sched = Sched(nc, bass)
            rings = [nc.sync, nc.scalar]

            # phase 1: input DMAs, one per proposal, alternating HWDGE
            # rings — fine arrival granularity so compute is never far
            # behind the data.
            di = 0
            for g in core["groups"]:
                for p in g["props"]:
                    rings[di % 2].dma_start(
                        out=xin_t[:, p.col:p.col + p.w],
                        in_=xin_d[:, p.col:p.col + p.w])
                    di += 1

            # phase 2: compute + output DMAs
            ngroups = len(core["groups"])
            for gi, g in enumerate(core["groups"]):
                late_g = gi >= ngroups - 2
                props = g["props"]
                gsz = len(props)
                L3 = g["L3"]
                lenA = props[0].lenA
                vb = props[0].variant == "B"
                grp_t = grp_pool.tile([128, gsz * 49 * lenA],
                                      mybir.dt.bfloat16, tag="grp",
                                      name=f"grp{gi}")

                j = 0
                while j < gsz:
                    p = props[j]
                    # run of consecutive members with L1!=7, L2!=7 and the
                    # same L2: share one x1 tile, batch stage 2 over all k.
                    if p.L1 != R and p.L2 != R:
                        k = 1
                        while (j + k < gsz
                               and props[j + k].L1 != R
                               and props[j + k].L2 == p.L2):
                            k += 1
                    else:
                        k = 1
                    if k > 1:
                        pitch = R * p.L2 * lenA
                        x1_t = x1_pool.tile([128, k * pitch],
                                            mybir.dt.bfloat16, tag="x1",
                                            name=f"x1_{gi}_{j}")
                        for m in range(k):
                            q = props[j + m]
                            emit_axis(sched,
                                      View.flat(x1_t, 0, 128, m * pitch,
                                                [R, q.L2 * lenA]),
                                      View.flat(xin_t, 0, 128, q.col,
                                                [q.L1, q.L2 * lenA]),
                                      0, axis_bins(q.L1))
                        if vb:
                            srcv = View.flat(x1_t, 0, 128, 0,
                                             [k * R, lenA, p.L2])
                            dstv = View.flat(grp_t, 0, 128, j * 49 * lenA,
                                             [k * R, lenA, R])
                            sdim = 2
                        else:
                            srcv = View.flat(x1_t, 0, 128, 0,
                                             [k * R, p.L2, lenA])
                            dstv = View.flat(grp_t, 0, 128, j * 49 * lenA,
                                             [k * R, R, lenA])
                            sdim = 1
                        emit_axis(sched, dstv, srcv, sdim, axis_bins(p.L2))
                        j += k
                        continue
                    src = View.flat(xin_t, 0, 128, p.col,
                                    [p.L1, p.L2 * lenA])
                    # group-slot dst: A = [7, 7(a2), lenA]; B = [7, lenA, 7(a2)]
                    if vb:
                        dst_g = View.flat(grp_t, 0, 128, j * 49 * lenA,
                                          [R, lenA, R])
                        x1_dims = [R, lenA, p.L2]
                        sdim = 2
                    else:
                        dst_g = View.flat(grp_t, 0, 128, j * 49 * lenA,
                                          [R, R, lenA])
                        x1_dims = [R, p.L2, lenA]
                        sdim = 1
                    if p.L1 == R and p.L2 == R:
                        # identity through both stages: copy into group tile
                        sched.copy(dst_g,
                                   View.flat(xin_t, 0, 128, p.col, x1_dims),
                                   late=late_g)
                        j += 1
                        continue
                    if p.L1 == R:
                        x1v = View.flat(xin_t, 0, 128, p.col, x1_dims)
                    elif p.L2 == R:
                        # stage 1 writes straight into the group tile
                        emit_axis(sched,
                                  View.flat(grp_t, 0, 128, j * 49 * lenA,
                                            [R, R * lenA]),
                                  src, 0, axis_bins(p.L1))
                        j += 1
                        continue
                    else:
                        x1_t = x1_pool.tile([128, R * p.L2 * lenA],
                                            mybir.dt.bfloat16, tag="x1",
                                            name=f"x1_{gi}_{j}")
                        emit_axis(sched,
                                  View.flat(x1_t, 0, 128, 0,
                                            [R, p.L2 * lenA]),
                                  src, 0, axis_bins(p.L1))
                        x1v = View.flat(x1_t, 0, 128, 0, x1_dims)
                    # stage 2 (L2 != 7 here) -> grp
                    emit_axis(sched, dst_g, x1v, sdim, axis_bins(p.L2))
                    j += 1

                # stage 3: pool lenA dim; A half bins 0-3, B half bins 3-6
                if L3 == R:
                    go_t = grp_t          # lenA == 4, identity
                else:
                    go_t = go_pool.tile([128, gsz * 49 * 4],
                                        mybir.dt.bfloat16, tag="go",
                                        name=f"go{gi}")
                    if vb:                # [g*7, lenA, 7] -> [g*7, 4, 7]
                        sdims = [gsz * R, lenA, R]
                        ddims = [gsz * R, 4, R]
                        pdim = 1
                    else:                 # [g*49, lenA] -> [g*49, 4]
                        sdims = [gsz * 49, lenA]
                        ddims = [gsz * 49, 4]
                        pdim = 1
                    opsA = [(kind, run) for kind, runs in
                            axis_ops(props[0].binsA) for run in runs]
                    opsB = [(kind, run) for kind, runs in
                            axis_ops(props[0].binsB) for run in runs]
                    both = set(opsA) & set(opsB)
                    srcA = View.flat(grp_t, 0, 64, 0, sdims)
                    srcB = View.flat(grp_t, 64, 128, 0, sdims)
                    srcF = View.flat(grp_t, 0, 128, 0, sdims)
                    dstA = View.flat(go_t, 0, 64, 0, ddims)
                    dstB = View.flat(go_t, 64, 128, 0, ddims)
                    dstF = View.flat(go_t, 0, 128, 0, ddims)

                    def emit_ops(ops, dst, src):
                        for kind, (i0, s0, di, ds, cnt) in ops:
                            dv = dst.slice(pdim, i0, di if cnt > 1 else 1,
                                           cnt)
                            if kind == "copy":
                                sched.copy(dv, src.slice(pdim, s0, ds, cnt), late=late_g)
                            else:
                                sched.tmax(dv, src.slice(pdim, s0, ds, cnt),
                                           src.slice(pdim, s0 + 1, ds, cnt))
                                if kind == "max3":
                                    sched.tmax(dv, dv,
                                               src.slice(pdim, s0 + 2, ds,
                                                         cnt))

                    emit_ops(sorted(both), dstF, srcF)
                    emit_ops([o for o in opsA if o not in both], dstA, srcA)
                    emit_ops([o for o in opsB if o not in both], dstB, srcB)

                s0 = g["slot0"]
                rings[gi % 2].dma_start(
                    out=out_d[s0:s0 + gsz].transpose([1, 0, 2]),
                    in_=View.flat(go_t, 0, 128, 0, [gsz, 196]).ap(bass))

    nc.compile()
    return nc


TRACE = False
LAST_RESULTS = None


def kernel(f, inputs, proposals, cls_ind):
    f16 = np.asarray(f, dtype=np.float32).astype(BF16)
    cores = plan(proposals)

    programs = []
    for core in cores:
        if not core["nslots"]:
            programs.append(None)
            continue
        nc = build_core_program(core)
        xin = pack_core(f16, core)
        programs.append((nc, {"xin": xin}, core))

    results = _run_programs(programs)

    out = np.zeros((N, C, R, R, R), np.float32)
    for prog, res in zip(programs, results):
        if prog is None:
            continue
        _, _, core = prog
        ns = core["nslots"]
        out_dev = np.asarray(res["out"]).reshape(ns, 128, 196)
        unpack_core(core, out_dev, out)
    return out


def _run_programs(programs):
    import jax
    from concourse.bass_utils import run_bass_kernel_spmd

    global LAST_RESULTS
    devices = jax.devices()
    results = []
    raw = []
    for c, prog in enumerate(programs):
        if prog is None:
            results.append(None)
            raw.append(None)
            continue
        nc, in_map, _ = prog
        with jax.default_device(devices[c % len(devices)]):
            res = run_bass_kernel_spmd(nc, [in_map], core_ids=[0],
                                       trace=TRACE)
        raw.append(res)
        results.append(res.results[0])
    LAST_RESULTS = raw
    return results

